# revision 25
# baseline (speedup 1.0000x reference)
"""Trainium2 Bass kernel for MemoryEfficientPatchDownScale.

Reference computation (per image):
  patchify 2x2 -> tokens (H/2*W/2, C*4)
  o1 = p @ W1.T + b1 ; o2 = silu(o1) ; o3 = o2 @ W2.T + b2
  out = o3 as (OUT_C, H/2, W/2) + repeat(avgpool2x2(x), 2, axis=C)

Strategy: data-parallel over batch across 8 NeuronCores (2 images/core).
Layout trick: SBUF partitions hold (c, s1) pairs (128 = 64 channels x 2
patch rows) so each partition's DMA line is a contiguous row of x.  The
s2 (even/odd pixel) half of the patch feature axis is handled by reading
the matmul moving operand with a stride-2 access pattern.  The residual
average-pool is folded into the second matmul's PSUM accumulation as an
extra K=128 matmul with a constant 0.25-valued selection matrix.
Matmul operands are bf16 (weights cast on host, x cast on-chip on the
vector engine, silu output written as bf16 by the scalar engine);
accumulation is fp32 in PSUM and the output is exact fp32 plumbing.
"""

import numpy as np

# Problem constants (hardcoded per harness contract)
B, C, H, W = 16, 64, 256, 256
S = 2
HIDDEN = 512
OUT_C = 128
N_CORES = 8
BC = B // N_CORES  # images per core

_NC_CACHE = {}


def build_nc(bc=BC, h=H, w=W, mm_dtype="bfloat16", nh2=4, act="Silu",
             dma_mode="s1cast", xin=3, xcast=3, o2b=2, outp=3, pipe=1, ps2b=3,
             resmode="dve", ps1b0=1, cast_eng="vector", finish_first=False):
    """Build the per-core Bass program. Token group = nh2 rows of w//2 tokens."""
    key = (bc, h, w, mm_dtype, nh2, act, dma_mode, xin, xcast, o2b, outp, pipe, ps2b, resmode, ps1b0, cast_eng, finish_first)
    if key in _NC_CACHE:
        return _NC_CACHE[key]
    from concourse import bacc
    import concourse.mybir as mybir
    import concourse.tile as tile

    f32 = mybir.dt.float32
    mmdt = getattr(mybir.dt, mm_dtype)
    cast = mmdt != f32
    h2, w2 = h // S, w // S
    ngroups = h2 // nh2
    ntok = nh2 * w2  # tokens per group == matmul free dim
    assert ntok <= 512

    nc = bacc.Bacc(None, target_bir_lowering=False)
    x = nc.dram_tensor("x", (bc, C, h, w), f32, kind="ExternalInput")
    w1 = nc.dram_tensor("w1", (128, 2, HIDDEN), mmdt, kind="ExternalInput")
    w2t = nc.dram_tensor("w2", (128, 4, OUT_C), mmdt, kind="ExternalInput")
    rw = nc.dram_tensor("rw", (128, OUT_C), mmdt, kind="ExternalInput")
    b1 = nc.dram_tensor("b1", (128, 4), f32, kind="ExternalInput")
    b2 = nc.dram_tensor("b2", (128, 1), f32, kind="ExternalInput")
    out = nc.dram_tensor("out", (bc, OUT_C, h2 * w2), f32, kind="ExternalOutput")

    # view: x[b, c, 2*h2+s1, w] -> xv[b, c, s1, h2, w]
    xv = x.rearrange("b c (hh s1) w -> b s1 c hh w", s1=2)

    actf = getattr(mybir.ActivationFunctionType, act)

    with tile.TileContext(nc) as tc:
        with (
            tc.tile_pool(name="const", bufs=1) as cpool,
            tc.tile_pool(name="xin", bufs=xin) as xpool,
            tc.tile_pool(name="xcast", bufs=xcast) as xbpool,
            tc.tile_pool(name="act", bufs=2) as apool,
            tc.tile_pool(name="outp", bufs=outp) as rpool,
            tc.tile_pool(name="xsum", bufs=2) as xspool,
            tc.tile_pool(name="ps1", bufs=1, space="PSUM") as ps1pool,
            tc.tile_pool(name="ps2", bufs=ps2b, space="PSUM") as ps2pool,
        ):
            w1t = cpool.tile([128, 2, HIDDEN], mmdt)
            w2tt = cpool.tile([128, 4, OUT_C], mmdt)
            rwt = cpool.tile([128, OUT_C], mmdt)
            b1t = cpool.tile([128, 4], f32)
            b2t = cpool.tile([128, 1], f32)
            nc.sync.dma_start(w1t[:], w1[:])
            nc.sync.dma_start(w2tt[:], w2t[:])
            nc.sync.dma_start(rwt[:], rw[:])
            nc.sync.dma_start(b1t[:], b1[:])
            nc.sync.dma_start(b2t[:], b2[:])

            def load_group(b, g):
                """DMA in one group of x rows and cast fp32 -> bf16 on DVE."""
                if dma_mode == "s1cast":
                    xt = None
                elif cast:
                    xt = xpool.tile([128, nh2, w], f32)
                else:
                    xt = xpool.tile([128, nh2, w], f32, tag="xb")
                if dma_mode == "s1":
                    for s1 in (0, 1):
                        nc.sync.dma_start(xt[s1 * 64:(s1 + 1) * 64, :, :],
                                          xv[b, s1, :, g * nh2:(g + 1) * nh2, :])
                elif dma_mode == "s1cast":
                    xb = xbpool.tile([128, nh2, w], mmdt)
                    for s1 in (0, 1):
                        nc.gpsimd.dma_start(xb[s1 * 64:(s1 + 1) * 64, :, :],
                                            xv[b, s1, :, g * nh2:(g + 1) * nh2, :])
                    return xb
                else:
                    for j in range(nh2):
                        nc.sync.dma_start(xt[:, j, :], xv[b, :, :, g * nh2 + j, :])
                if cast:
                    xb = xbpool.tile([128, nh2, w], mmdt)
                    getattr(nc, cast_eng).tensor_copy(xb[:], xt[:])
                    return xb
                return xt

            def finish_group(st):
                """Second matmul + evacuation for a group whose silu is done."""
                b, g, ps2, o2s = st
                for kt in range(4):
                    nc.tensor.matmul(ps2[:], w2tt[:, kt, :], o2s[kt][:],
                                     start=False, stop=(kt == 3))
                ot = rpool.tile([128, ntok], f32)
                nc.vector.tensor_scalar_add(out=ot[:], in0=ps2[:], scalar1=b2t[:])
                nc.sync.dma_start(out[b, :, g * ntok:(g + 1) * ntok], ot[:])

            groups = [(b, g) for b in range(bc) for g in range(ngroups)]
            xbs = {0: load_group(*groups[0])}
            pending = []  # groups whose m2/evac is deferred (2-deep pipeline)
            for i, (b, g) in enumerate(groups):
                # prefetch next group's load+cast so the DVE cast is
                # emitted (and runs) before this group's PSUM drain
                if i + 1 < len(groups):
                    xbs[i + 1] = load_group(*groups[i + 1])
                if finish_first and len(pending) >= pipe:
                    finish_group(pending.pop(0))
                xb = xbs.pop(i)
                o2s = []
                for ht in range(4):
                    ps1 = ps1pool.tile([128, ntok], f32, tag=f"ps1_{ht}",
                                       bufs=(ps1b0 if ht == 0 else 1))
                    for s2 in (0, 1):
                        nc.tensor.matmul(
                            ps1[:], w1t[:, s2, ht * 128:(ht + 1) * 128],
                            xb[:, :, s2::2], start=(s2 == 0), stop=(s2 == 1))
                    o2t = apool.tile([128, ntok], mmdt, tag=f"o2_{ht}", bufs=o2b)
                    nc.scalar.activation(out=o2t[:], in_=ps1[:],
                                         func=actf, bias=b1t[:, ht:ht + 1], scale=1.0)
                    o2s.append(o2t)
                ps2 = ps2pool.tile([128, ntok], f32)
                # residual: 0.25 * sum over (s1, s2) of x patch, selected per out-channel
                if resmode == "dve":
                    xs = xspool.tile([128, nh2, w2], mmdt)
                    nc.vector.tensor_add(xs[:], xb[:, :, 0::2], xb[:, :, 1::2])
                    nc.tensor.matmul(ps2[:], rwt[:], xs[:], start=True, stop=False)
                else:
                    nc.tensor.matmul(ps2[:], rwt[:], xb[:, :, 0::2], start=True, stop=False)
                    nc.tensor.matmul(ps2[:], rwt[:], xb[:, :, 1::2], start=False, stop=False)
                pending.append((b, g, ps2, o2s))
                if not finish_first and len(pending) > pipe:
                    finish_group(pending.pop(0))
            for st in pending:
                finish_group(st)

    nc.compile()
    _NC_CACHE[key] = nc
    return nc


def prep_weights(weight1, bias1, weight2, bias2, mm_dtype="bfloat16"):
    """Host-side weight relayout. Feature index k = c*4 + s1*2 + s2;
    SBUF partition p = s1*64 + c."""
    if mm_dtype == "bfloat16":
        import ml_dtypes
        wdt = ml_dtypes.bfloat16
    else:
        wdt = np.float32
    w1c = np.ascontiguousarray(
        np.asarray(weight1, np.float32)
        .reshape(HIDDEN, C, 2, 2).transpose(2, 1, 3, 0).reshape(128, 2, HIDDEN)
    ).astype(wdt)
    w2c = np.ascontiguousarray(
        np.asarray(weight2, np.float32).T.reshape(4, 128, OUT_C).transpose(1, 0, 2)
    ).astype(wdt)
    rwc = np.zeros((128, OUT_C), np.float32)
    oc = np.arange(OUT_C)
    rwc[oc // 2, oc] = 0.25
    rwc[64 + oc // 2, oc] = 0.25
    rwc = rwc.astype(wdt)
    b1c = np.ascontiguousarray(np.asarray(bias1, np.float32).reshape(4, 128).T)
    b2c = np.ascontiguousarray(np.asarray(bias2, np.float32).reshape(OUT_C, 1))
    return w1c, w2c, rwc, b1c, b2c


def kernel(x, weight1, bias1, weight2, bias2):
    from concourse.bass_utils import run_bass_kernel_spmd

    x = np.asarray(x, dtype=np.float32)
    w1c, w2c, rwc, b1c, b2c = prep_weights(weight1, bias1, weight2, bias2)
    nc = build_nc()
    in_maps = [
        {"x": np.ascontiguousarray(x[i * BC:(i + 1) * BC]),
         "w1": w1c, "w2": w2c, "rw": rwc, "b1": b1c, "b2": b2c}
        for i in range(N_CORES)
    ]
    res = run_bass_kernel_spmd(nc, in_maps, core_ids=list(range(N_CORES)))
    outs = [r["out"].reshape(BC, OUT_C, H // S, W // S) for r in res.results]
    return np.concatenate(outs, axis=0)
